# revision 1
# baseline (speedup 1.0000x reference)
"""STFT (Conv1D-style) Bass kernel for Trainium2, 8 NeuronCores.

Strategy (data-parallel over batch):
  - B=16 signals, 8 cores -> 2 signals per core.
  - Host: reflect-pad, cast to bf16, and lay the padded signal out as
    128-sample blocks transposed to [128, nblocks] (partition = offset
    within block, column = block index). Because HOP=256 = 2*128, frame t
    chunk c (128 samples starting at 256t+128c) is exactly block 2t+c, so
    the stationary matmul operand for an M-tile of frames is a stride-2
    column slice of this layout -- no on-device gather/transpose needed.
  - Host: build the windowed DFT basis Bc[n, 2f] = cos(2pi k n/N)*w[n],
    Bc[n, 2f+1] = -sin(2pi k n/N)*w[n]  ([1024, 1026], bf16), so one
    matmul produces the interleaved (real, imag) output layout directly.
  - Device: out[t, f2] = sum_n frames[t, n] * Bc[n, f2] as 8 accumulating
    K-chunk matmuls (K=128) per (M-tile of 128 frames, N-chunk of <=512).
  - Gather per-core [2, 1876, 1026] f32 outputs -> [16, 1876, 513, 2].
"""

import numpy as np
import ml_dtypes

N_FFT = 1024
HOP = 256
B = 16
T = 480000
F = N_FFT // 2 + 1          # 513
F2 = 2 * F                  # 1026
PAD = N_FFT // 2            # 512
XP_LEN = T + 2 * PAD        # 481024
NBLK = XP_LEN // 128        # 3758
NBLK_PAD = 3760             # padded to a multiple of 16 blocks
NF = (XP_LEN - N_FFT) // HOP + 1   # 1876 frames
NCORES = 8
B_PER_CORE = B // NCORES    # 2
# Device computes k=0..511 (1024 interleaved re/im columns); the k=512
# (Nyquist) pair is 0.1% of the FLOPs and would cost 240 tiny N=2 matmuls
# (~30us of unhidden weight loads), so it is done on host BLAS instead.
F2D = 1024
N_SPLIT = [(0, 512), (512, 512)]

_CACHE = {}


MODE = "fp32r"          # "bf16" | "fp32r"


def _in_dt(mybir):
    return mybir.dt.bfloat16 if MODE == "bf16" else mybir.dt.float32r


def _build_nc(repeat=1):
    import concourse.mybir as mybir
    import concourse.tile as tile
    from concourse import bacc

    idt = _in_dt(mybir)
    nc = bacc.Bacc("TRN2", target_bir_lowering=False, debug=False,
                   num_devices=NCORES)
    xpt = nc.dram_tensor("xpt", [128, B_PER_CORE, NBLK_PAD // 2, 2],
                         idt, kind="ExternalInput")
    basis = nc.dram_tensor("basis", [128, 8, F2D], idt,
                           kind="ExternalInput")
    out = nc.dram_tensor("out", [B_PER_CORE, NF, F2D], mybir.dt.float32,
                         kind="ExternalOutput")

    with tile.TileContext(nc) as tc:
        with (
            tc.tile_pool(name="sig", bufs=min(repeat, 2)) as sigp,
            tc.tile_pool(name="bas", bufs=min(repeat, 2)) as basp,
            tc.tile_pool(name="outp", bufs=3) as outp,
            tc.tile_pool(name="ps", bufs=2, space="PSUM") as psp,
        ):
            for _rep in range(repeat):
                sig = sigp.tile([128, B_PER_CORE, NBLK_PAD // 2, 2], idt,
                                name="sig", tag="sig")
                bas = basp.tile([128, 8, F2D], idt, name="bas", tag="bas")
                # chunked input DMAs so the first matmuls only wait for the
                # slices they read (basis chunk c=0/1 + batch-0 signal first)
                nc.sync.dma_start(bas[:, 0:2], basis[:, 0:2])
                nc.sync.dma_start(sig[:, 0], xpt[:, 0])
                nc.sync.dma_start(bas[:, 2:4], basis[:, 2:4])
                nc.sync.dma_start(bas[:, 4:6], basis[:, 4:6])
                nc.sync.dma_start(bas[:, 6:8], basis[:, 6:8])
                nc.sync.dma_start(sig[:, 1], xpt[:, 1])

                for b in range(B_PER_CORE):
                    for t0 in range(0, NF, 128):
                        M = min(128, NF - t0)
                        pss = [
                            psp.tile([128, w], mybir.dt.float32, tag=f"ps{i}",
                                     name=f"ps{i}")
                            for i, (_, w) in enumerate(N_SPLIT)
                        ]
                        for c in range(8):
                            q, r = divmod(c, 2)
                            lhsT = sig[:, b, t0 + q:t0 + q + M, r]
                            for i, (o, w) in enumerate(N_SPLIT):
                                nc.tensor.matmul(
                                    pss[i][:M, :], lhsT, bas[:, c, o:o + w],
                                    start=(c == 0), stop=(c == 7),
                                )
                        ot = outp.tile([128, F2D], mybir.dt.float32)
                        for i, (o, w) in enumerate(N_SPLIT):
                            nc.vector.tensor_copy(ot[:M, o:o + w],
                                                  pss[i][:M, :])
                        nc.sync.dma_start(out[b, t0:t0 + M, :], ot[:M, :])

    nc.compile()
    return nc


def _host_prep(x, window):
    xp = np.pad(x.astype(np.float32), ((0, 0), (PAD, PAD)), mode="reflect")
    xp = np.concatenate(
        [xp, np.zeros((B, NBLK_PAD * 128 - XP_LEN), np.float32)], axis=1)
    # [B, nblk, 128] -> [B, 128, nblk]
    xpt = np.ascontiguousarray(xp.reshape(B, NBLK_PAD, 128).transpose(0, 2, 1))
    np_dt = ml_dtypes.bfloat16 if MODE == "bf16" else np.float32
    xpt_bf = xpt.astype(np_dt)

    # Windowed DFT basis, computed in fp32 to match the reference math.
    k = np.arange(F, dtype=np.float32)[:, None]
    n = np.arange(N_FFT, dtype=np.float32)[None, :]
    ang = np.float32(2.0 * np.pi / N_FFT) * k * n        # [F, N] fp32
    w32 = window.astype(np.float32)
    cosk = np.cos(ang) * w32[None, :]
    sink = -np.sin(ang) * w32[None, :]
    Bc = np.empty((N_FFT, F2), np.float32)
    Bc[:, 0::2] = cosk.T
    Bc[:, 1::2] = sink.T
    basis_sb = np.ascontiguousarray(
        Bc[:, :F2D].reshape(8, 128, F2D).transpose(1, 0, 2)).astype(np_dt)

    # k=512 (Nyquist) re/im pair on host BLAS (fp32, exact-grade).
    ny_basis = np.ascontiguousarray(Bc[:, F2D:])          # [1024, 2]
    nyq = np.empty((B, NF, 2), np.float32)
    for b in range(B):
        frames = np.lib.stride_tricks.as_strided(
            xp[b], (NF, N_FFT), (HOP * 4, 4))
        nyq[b] = frames @ ny_basis

    in_maps = []
    for c in range(NCORES):
        xc = xpt_bf[B_PER_CORE * c:B_PER_CORE * (c + 1)]   # [2, 128, 3760]
        xc = np.ascontiguousarray(xc.transpose(1, 0, 2)).reshape(
            128, B_PER_CORE, NBLK_PAD // 2, 2)
        in_maps.append({"xpt": xc, "basis": basis_sb})
    return in_maps, nyq


def kernel(x, window):
    from concourse.bass_utils import run_bass_kernel_spmd

    if "nc" not in _CACHE:
        _CACHE["nc"] = _build_nc()
    nc = _CACHE["nc"]

    in_maps, nyq = _host_prep(np.asarray(x), np.asarray(window))
    res = run_bass_kernel_spmd(nc, in_maps, core_ids=list(range(NCORES)),
                               trace=False)
    dev = np.concatenate([res.results[c]["out"] for c in range(NCORES)],
                         axis=0)                     # [16, 1876, 1024]
    out = np.empty((B, NF, F, 2), np.float32)
    out[:, :, :F - 1, :] = dev.reshape(B, NF, F - 1, 2)
    out[:, :, F - 1, :] = nyq
    return out



# revision 4
# speedup vs baseline: 68482.6527x; 68482.6527x over previous
"""STFT (Conv1D-style) Bass kernel for Trainium2, 8 NeuronCores.

Radix-4 decimation-in-time restructure (4x fewer MACs than direct DFT):
  - Split each frame's 1024 samples into 4 streams by n mod 4. With the
    window and all twiddles folded into per-stream bases (= rows n≡c mod 4
    of the original windowed DFT basis, k=0..127 only), compute
      S_c(k) = sum_m x[4m+c] w[4m+c] W^(k(4m+c)),  k=0..127, c=0..3.
  - Full 513-freq spectrum via S_c(k+256) = (-i)^c S_c(k) and conjugate
    symmetry; all recombines are pure add/sub of S tiles:
      P02=S0+S2, M02=S0-S2, P13=S1+S3, M13=S1-S3 (re/im each)
      a0[p] = X_p         : re=P02r+P13r         im=P02i+P13i
      a1[q] = X_{256-q}   : re=M02r-M13i         im=-M02i-M13r
      a2[p] = X_{256+p}   : re=M02r+M13i         im=M02i-M13r
      a3[q] = X_{512-q}   : re=P02r-P13r         im=P13i-P02i
    k=128 and k=384 are the only freqs not covered; host BLAS computes them.
  - Layout: freqs on psum partitions, frames on the free (moving) dim.
    HOP=256 = 64*4, so stream-c samples of frame t are xs_c[64t .. 64t+255]
    (xs_c[j] = xp[4j+c]); blocked [128 x cols] layouts give frame tiles as
    contiguous column slices (separate layouts for even/odd frames since
    frames advance by half a block in stream index).
  - Data-parallel over batch: 2 signals per core; bf16 operands/outputs
    (fp32 psum accumulate), host upcasts.
"""

import numpy as np
import ml_dtypes

N_FFT = 1024
HOP = 256
B = 16
T = 480000
F = N_FFT // 2 + 1          # 513
PAD = N_FFT // 2            # 512
XP_LEN = T + 2 * PAD        # 481024
NF = (XP_LEN - N_FFT) // HOP + 1   # 1876 frames
NCORES = 8
B_PER_CORE = B // NCORES    # 2
NFE = NF // 2               # 938 frames per parity
NBC = 941                   # stream-layout columns (r+q <= 937+3, +zeros pad)
XS_LEN = XP_LEN // 4        # 120256 samples per stream
FT_SIZES = [256, 256, 256, 170]     # frame tiles per parity (sum=938)
HOST_KS = [128, 384]        # freqs computed on host BLAS

_CACHE = {}


def _build_nc(repeat=1):
    import concourse.mybir as mybir
    import concourse.tile as tile
    from concourse import bacc

    idt = mybir.dt.bfloat16
    f32 = mybir.dt.float32
    add = mybir.AluOpType.add
    sub = mybir.AluOpType.subtract
    mult = mybir.AluOpType.mult

    nc = bacc.Bacc("TRN2", target_bir_lowering=False, debug=False,
                   num_devices=NCORES)
    sig = nc.dram_tensor("sig", [128, B_PER_CORE, 2, 4, NBC], idt,
                         kind="ExternalInput")
    basis = nc.dram_tensor("basis", [128, 16, 128], idt,
                           kind="ExternalInput")
    out = nc.dram_tensor("out", [B_PER_CORE, 2, 8, 128, NFE], idt,
                         kind="ExternalOutput")

    with tile.TileContext(nc) as tc:
        with (
            tc.tile_pool(name="sigp", bufs=min(repeat, 2)) as sigp,
            tc.tile_pool(name="basp", bufs=min(repeat, 2)) as basp,
            tc.tile_pool(name="intp", bufs=2) as intp,
            tc.tile_pool(name="outp", bufs=3) as outp,
            tc.tile_pool(name="ps", bufs=2, space="PSUM") as psp,
        ):
            for _rep in range(repeat):
                sg = sigp.tile([128, B_PER_CORE, 2, 4, NBC], idt,
                               name="sg", tag="sg")
                bs = basp.tile([128, 16, 128], idt, name="bs", tag="bs")
                nc.sync.dma_start(bs[:], basis[:])
                for b in range(B_PER_CORE):
                    for par in range(2):
                        nc.sync.dma_start(sg[:, b, par], sig[:, b, par])

                for b in range(B_PER_CORE):
                    for par in range(2):
                        for ft, N in enumerate(FT_SIZES):
                            f0 = 256 * ft
                            # S_c tiles: [:, 0:N] = re, [:, 256:256+N] = im
                            # c order 2,3,0,1 so the ACT copies of S2/S3 can
                            # overlap the S0/S1 matmuls.
                            S = [psp.tile([128, 512], f32, tag=f"S{c}",
                                          name=f"S{c}") for c in range(4)]
                            for c in (2, 3, 0, 1):
                                for comp in range(2):
                                    po = S[c][:, 256 * comp:256 * comp + N]
                                    for q in range(2):
                                        ch = (c * 2 + comp) * 2 + q
                                        nc.tensor.matmul(
                                            po, bs[:, ch, :],
                                            sg[:, b, par, c, f0 + q:f0 + q + N],
                                            start=(q == 0), stop=(q == 1),
                                        )
                            Sr = [S[c][:, 0:N] for c in range(4)]
                            Si = [S[c][:, 256:256 + N] for c in range(4)]

                            it = {
                                nm: intp.tile([128, 256], f32, tag=nm,
                                              name=nm)[:, :N]
                                for nm in ("C2r", "C2i", "C3r", "C3i",
                                           "P02r", "M02r", "P13r", "M13r",
                                           "P02i", "M02i", "P13i", "M13i")
                            }
                            gp, ve, sc = nc.gpsimd, nc.vector, nc.scalar
                            # ACT: psum -> sbuf copies of streams 2, 3
                            sc.copy(it["C2r"], Sr[2])
                            sc.copy(it["C2i"], Si[2])
                            sc.copy(it["C3r"], Sr[3])
                            sc.copy(it["C3i"], Si[3])
                            # DVE: one psum + one sbuf operand each
                            ve.tensor_tensor(it["P02r"], Sr[0], it["C2r"], add)
                            ve.tensor_tensor(it["M02r"], Sr[0], it["C2r"], sub)
                            ve.tensor_tensor(it["P02i"], Si[0], it["C2i"], add)
                            ve.tensor_tensor(it["M02i"], Si[0], it["C2i"], sub)
                            ve.tensor_tensor(it["P13r"], Sr[1], it["C3r"], add)
                            ve.tensor_tensor(it["M13r"], Sr[1], it["C3r"], sub)
                            ve.tensor_tensor(it["P13i"], Si[1], it["C3i"], add)
                            ve.tensor_tensor(it["M13i"], Si[1], it["C3i"], sub)

                            ot = [outp.tile([128, 256], idt, tag=f"o{i}",
                                            name=f"o{i}")[:, :N]
                                  for i in range(8)]
                            # a0i, a1i on DVE (stt not supported on gpsimd)
                            ve.tensor_tensor(ot[1], it["P02i"], it["P13i"], add)
                            ve.scalar_tensor_tensor(
                                ot[3], it["M02i"], -1.0, it["M13r"], mult, sub)
                            # GpSimd (sbuf only, plain tensor_tensor)
                            gp.tensor_tensor(ot[0], it["P02r"], it["P13r"], add)
                            gp.tensor_tensor(ot[2], it["M02r"], it["M13i"], sub)
                            gp.tensor_tensor(ot[4], it["M02r"], it["M13i"], add)
                            gp.tensor_tensor(ot[5], it["M02i"], it["M13r"], sub)
                            gp.tensor_tensor(ot[6], it["P02r"], it["P13r"], sub)
                            gp.tensor_tensor(ot[7], it["P13i"], it["P02i"], sub)

                            for i in range(8):
                                nc.sync.dma_start(
                                    out[b, par, i, :, f0:f0 + N], ot[i])

    nc.compile()
    return nc


def _host_prep(x, window):
    x = np.asarray(x, dtype=np.float32)
    window = np.asarray(window, dtype=np.float32)
    xp = np.pad(x, ((0, 0), (PAD, PAD)), mode="reflect")

    # stream layouts: [B, 2par, 4c, 128, NBC] bf16
    lay = np.zeros((B, 2, 4, 128, NBC), np.float32)
    for c in range(4):
        xs = xp[:, c::4]                              # [B, 120256]
        xs_pad = np.zeros((B, 64 + NBC * 128), np.float32)
        xs_pad[:, :XS_LEN] = xs
        lay[:, 0, c] = xs_pad[:, :NBC * 128].reshape(B, NBC, 128).transpose(0, 2, 1)
        lay[:, 1, c] = xs_pad[:, 64:64 + NBC * 128].reshape(B, NBC, 128).transpose(0, 2, 1)
    lay_bf = lay.astype(ml_dtypes.bfloat16)

    # Windowed DFT basis, fp32 angles to match the reference math.
    k = np.arange(F, dtype=np.float32)[:, None]
    n = np.arange(N_FFT, dtype=np.float32)[None, :]
    ang = np.float32(2.0 * np.pi / N_FFT) * k * n
    cosk = np.cos(ang) * window[None, :]              # [F, 1024]
    sink = -np.sin(ang) * window[None, :]
    Bre = cosk.T.astype(np.float32)                   # [1024, F]
    Bim = sink.T.astype(np.float32)

    bas_host = np.empty((128, 16, 128), np.float32)
    p = np.arange(128)
    for c in range(4):
        for comp in range(2):
            src = Bre if comp == 0 else Bim
            for q in range(2):
                ch = (c * 2 + comp) * 2 + q
                bas_host[:, ch, :] = src[512 * q + 4 * p + c, :128]
    bas_bf = bas_host.astype(ml_dtypes.bfloat16)

    # host freqs k=128, 384 (fp32 BLAS)
    hb = np.stack([Bre[:, HOST_KS[0]], Bim[:, HOST_KS[0]],
                   Bre[:, HOST_KS[1]], Bim[:, HOST_KS[1]]], axis=1)  # [1024,4]
    hout = np.empty((B, NF, 4), np.float32)
    for b in range(B):
        frames = np.lib.stride_tricks.as_strided(
            xp[b], (NF, N_FFT), (HOP * 4, 4))
        hout[b] = frames @ hb

    in_maps = []
    for core in range(NCORES):
        bs = slice(B_PER_CORE * core, B_PER_CORE * (core + 1))
        # [128, B_PER_CORE, 2, 4, NBC]
        sc = np.ascontiguousarray(lay_bf[bs].transpose(3, 0, 1, 2, 4))
        in_maps.append({"sig": sc, "basis": bas_bf})
    return in_maps, hout


def _assemble(results, prep):
    _, hout = prep
    dev = np.concatenate(
        [np.asarray(results.results[c]["out"]) for c in range(NCORES)],
        axis=0).astype(np.float32)                    # [B, 2, 8, 128, NFE]
    out = np.empty((B, NF, F, 2), np.float32)
    idx1 = 256 - np.arange(128)
    idx3 = 512 - np.arange(128)
    for par in range(2):
        A = dev[:, par].transpose(0, 3, 1, 2)         # [B, NFE, 8, 128]
        ov = out[:, par::2]                           # view [B, NFE, F, 2]
        ov[:, :, 0:128, 0] = A[:, :, 0]
        ov[:, :, 0:128, 1] = A[:, :, 1]
        ov[:, :, idx1, 0] = A[:, :, 2]
        ov[:, :, idx1, 1] = A[:, :, 3]
        ov[:, :, 256:384, 0] = A[:, :, 4]
        ov[:, :, 256:384, 1] = A[:, :, 5]
        ov[:, :, idx3, 0] = A[:, :, 6]
        ov[:, :, idx3, 1] = A[:, :, 7]
    out[:, :, HOST_KS[0], 0] = hout[:, :, 0]
    out[:, :, HOST_KS[0], 1] = hout[:, :, 1]
    out[:, :, HOST_KS[1], 0] = hout[:, :, 2]
    out[:, :, HOST_KS[1], 1] = hout[:, :, 3]
    return out


def kernel(x, window):
    from concourse.bass_utils import run_bass_kernel_spmd

    if "nc" not in _CACHE:
        _CACHE["nc"] = _build_nc()
    nc = _CACHE["nc"]

    prep = _host_prep(np.asarray(x), np.asarray(window))
    res = run_bass_kernel_spmd(nc, prep[0], core_ids=list(range(NCORES)),
                               trace=False)
    return _assemble(res, prep)


# revision 5
# speedup vs baseline: 83903.5094x; 1.2252x over previous
"""STFT (Conv1D-style) Bass kernel for Trainium2, 8 NeuronCores.

Radix-2 decimation-in-time restructure (2x fewer MACs than direct DFT):
  - Split each frame's 1024 samples into even/odd streams. With window and
    twiddles folded into the bases (= even/odd rows of the original windowed
    DFT basis, k=0..255 only):
      E(k) = sum_m x[2m] w[2m] W^(2mk),  O(k) = sum_m x[2m+1] w[2m+1] W^((2m+1)k)
  - Then X_k = E+O for k=0..255 and X_{512-k} = conj(E-O):
    combine is ONE psum->sbuf copy + add + sub per tile; host negates the
    upper block's imag half and computes the single missing freq k=256.
  - Layout: freqs on psum partitions (2 blocks of 128), frames on the
    moving dim. HOP=256 = 2*128, so stream samples of frame t are
    xs[128t .. 128t+511]; blocked [128 x 1879] layouts make every frame
    tile a contiguous column slice (no parity split needed).
  - Data-parallel over batch: 2 signals per core; bf16 operands/outputs
    (fp32 psum accumulate), host upcasts. Combine alternates between a
    DVE path (ACT copies O, DVE adds from psum) and a GP path (ACT copies
    both, GpSimd adds in sbuf) to spread load over all vector engines.
"""

import numpy as np
import ml_dtypes

N_FFT = 1024
HOP = 256
B = 16
T = 480000
F = N_FFT // 2 + 1          # 513
PAD = N_FFT // 2            # 512
XP_LEN = T + 2 * PAD        # 481024
NF = (XP_LEN - N_FFT) // HOP + 1   # 1876 frames
NCORES = 8
B_PER_CORE = B // NCORES    # 2
NBC = 1879                  # stream-layout columns = 240512 / 128
XS_LEN = XP_LEN // 2        # 240512 samples per stream
FT_SIZES = [512, 512, 512, 340]     # frame tiles (sum = 1876)
HOST_KS = [256]             # freqs computed on host BLAS

_CACHE = {}


def _build_nc(repeat=1):
    import concourse.mybir as mybir
    import concourse.tile as tile
    from concourse import bacc

    idt = mybir.dt.bfloat16
    f32 = mybir.dt.float32
    add = mybir.AluOpType.add
    sub = mybir.AluOpType.subtract

    nc = bacc.Bacc("TRN2", target_bir_lowering=False, debug=False,
                   num_devices=NCORES)
    sig = nc.dram_tensor("sig", [128, B_PER_CORE, 2, NBC], idt,
                         kind="ExternalInput")
    basis = nc.dram_tensor("basis", [128, 32, 128], idt,
                           kind="ExternalInput")
    out = nc.dram_tensor("out", [B_PER_CORE, 2, 2, 128, 2, NF], idt,
                         kind="ExternalOutput")

    with tile.TileContext(nc) as tc:
        with (
            tc.tile_pool(name="sigp", bufs=min(repeat, 2)) as sigp,
            tc.tile_pool(name="basp", bufs=min(repeat, 2)) as basp,
            tc.tile_pool(name="intp", bufs=2) as intp,
            tc.tile_pool(name="outp", bufs=3) as outp,
            tc.tile_pool(name="ps", bufs=2, space="PSUM") as psp,
        ):
            for _rep in range(repeat):
                sg = sigp.tile([128, B_PER_CORE, 2, NBC], idt,
                               name="sg", tag="sg")
                bs = basp.tile([128, 32, 128], idt, name="bs", tag="bs")
                nc.sync.dma_start(bs[:], basis[:])
                for b in range(B_PER_CORE):
                    for s in range(2):
                        nc.sync.dma_start(sg[:, b, s], sig[:, b, s])

                unit = 0
                for b in range(B_PER_CORE):
                    for ft, N in enumerate(FT_SIZES):
                        f0 = 512 * ft
                        for g in range(2):
                            # E/O psum tiles [128, 2(comp), 512]
                            E = psp.tile([128, 2, 512], f32, tag="E", name="E")
                            O = psp.tile([128, 2, 512], f32, tag="O", name="O")
                            for s, pt in ((0, E), (1, O)):
                                for comp in range(2):
                                    po = pt[:, comp, 0:N]
                                    for q in range(4):
                                        ch = ((s * 2 + comp) * 2 + g) * 4 + q
                                        nc.tensor.matmul(
                                            po, bs[:, ch, :],
                                            sg[:, b, s, f0 + q:f0 + q + N],
                                            start=(q == 0), stop=(q == 3),
                                        )
                            Ev, Ov = E[:, :, 0:N], O[:, :, 0:N]
                            xl = outp.tile([128, 2, 512], idt, tag="xl",
                                           name="xl")[:, :, 0:N]
                            xu = outp.tile([128, 2, 512], idt, tag="xu",
                                           name="xu")[:, :, 0:N]
                            if unit % 2 == 0:
                                co = intp.tile([128, 2, 512], f32, tag="co",
                                               name="co")[:, :, 0:N]
                                nc.scalar.copy(co, Ov)
                                nc.vector.tensor_tensor(xl, Ev, co, add)
                                nc.vector.tensor_tensor(xu, Ev, co, sub)
                            else:
                                ce = intp.tile([128, 2, 512], f32, tag="ce",
                                               name="ce")[:, :, 0:N]
                                co = intp.tile([128, 2, 512], f32, tag="co2",
                                               name="co2")[:, :, 0:N]
                                nc.scalar.copy(ce, Ev)
                                nc.scalar.copy(co, Ov)
                                nc.gpsimd.tensor_tensor(xl, ce, co, add)
                                nc.gpsimd.tensor_tensor(xu, ce, co, sub)
                            nc.sync.dma_start(
                                out[b, 0, g, :, :, f0:f0 + N], xl)
                            nc.sync.dma_start(
                                out[b, 1, g, :, :, f0:f0 + N], xu)
                            unit += 1

    nc.compile()
    return nc


def _host_prep(x, window):
    x = np.asarray(x, dtype=np.float32)
    window = np.asarray(window, dtype=np.float32)
    xp = np.pad(x, ((0, 0), (PAD, PAD)), mode="reflect")

    # stream layouts [B, 2(stream), 128, NBC]
    lay = np.empty((B, 2, 128, NBC), np.float32)
    for s in range(2):
        xs = xp[:, s::2]                              # [B, 240512]
        lay[:, s] = xs.reshape(B, NBC, 128).transpose(0, 2, 1)
    lay_bf = lay.astype(ml_dtypes.bfloat16)

    # Windowed DFT basis, fp32 angles to match the reference math.
    k = np.arange(F, dtype=np.float32)[:, None]
    n = np.arange(N_FFT, dtype=np.float32)[None, :]
    ang = np.float32(2.0 * np.pi / N_FFT) * k * n
    cosk = np.cos(ang) * window[None, :]              # [F, 1024]
    sink = -np.sin(ang) * window[None, :]
    Bre = cosk.T.astype(np.float32)                   # [1024, F]
    Bim = sink.T.astype(np.float32)

    # chunk = ((s*2 + comp)*2 + g)*4 + q ; rows n = 2*(128q+p)+s
    bas_host = np.empty((128, 32, 128), np.float32)
    p = np.arange(128)
    for s in range(2):
        for comp in range(2):
            src = Bre if comp == 0 else Bim
            for g in range(2):
                for q in range(4):
                    ch = ((s * 2 + comp) * 2 + g) * 4 + q
                    bas_host[:, ch, :] = src[2 * (128 * q + p) + s,
                                             128 * g:128 * g + 128]
    bas_bf = bas_host.astype(ml_dtypes.bfloat16)

    # host freq k=256 (fp32 BLAS)
    hb = np.stack([Bre[:, 256], Bim[:, 256]], axis=1)  # [1024, 2]
    hout = np.empty((B, NF, 2), np.float32)
    for b in range(B):
        frames = np.lib.stride_tricks.as_strided(
            xp[b], (NF, N_FFT), (HOP * 4, 4))
        hout[b] = frames @ hb

    in_maps = []
    for core in range(NCORES):
        bsl = slice(B_PER_CORE * core, B_PER_CORE * (core + 1))
        sc = np.ascontiguousarray(lay_bf[bsl].transpose(2, 0, 1, 3))
        in_maps.append({"sig": sc, "basis": bas_bf})
    return in_maps, hout


def _assemble(results, prep):
    _, hout = prep
    dev = np.concatenate(
        [np.asarray(results.results[c]["out"]) for c in range(NCORES)],
        axis=0).astype(np.float32)            # [B, 2lu, 2g, 128, 2comp, NF]
    out = np.empty((B, NF, F, 2), np.float32)
    j = np.arange(128)
    for g in range(2):
        lo = dev[:, 0, g].transpose(0, 3, 1, 2)       # [B, NF, 128, 2]
        out[:, :, 128 * g:128 * g + 128, :] = lo
        up = dev[:, 1, g].transpose(0, 3, 1, 2)       # [B, NF, 128, 2]
        ks = 512 - (128 * g + j)                      # descending freqs
        out[:, :, ks, 0] = up[:, :, :, 0]
        out[:, :, ks, 1] = -up[:, :, :, 1]
    out[:, :, 256, 0] = hout[:, :, 0]
    out[:, :, 256, 1] = hout[:, :, 1]
    return out


def kernel(x, window):
    from concourse.bass_utils import run_bass_kernel_spmd

    if "nc" not in _CACHE:
        _CACHE["nc"] = _build_nc()
    nc = _CACHE["nc"]

    prep = _host_prep(np.asarray(x), np.asarray(window))
    res = run_bass_kernel_spmd(nc, prep[0], core_ids=list(range(NCORES)),
                               trace=False)
    return _assemble(res, prep)


# revision 8
# speedup vs baseline: 126613.2237x; 1.5090x over previous
"""STFT (Conv1D-style) Bass kernel for Trainium2, 8 NeuronCores.

Radix-4 decimation-in-time (4x fewer MACs than direct DFT), engineered
around TRN2's PSUM-read wall (only DVE/ACT read PSUM, ~112 G elem/s each):

  - 4 sample streams by n mod 4; window + all twiddles folded into
    per-stream bases (= rows n≡c mod 4 of the windowed DFT basis, k<=127):
      S_c(k) = sum_m x[4m+c] w[4m+c] W^(k(4m+c)),  k=0..127.
  - Full spectrum from S_c(k+256) = (-i)^c S_c(k) + conjugate symmetry;
    every recombine is a pure add/sub.  Each PSUM value is read EXACTLY
    once: ACT copies 2*S2, 2*S3 to sbuf bf16; DVE forms P = 0.5*C + S0/S1
    (one fused scalar_tensor_tensor, the only psum read); GpSimd forms
    M' = C - P in sbuf.  Stage 2 (8 adds/subs per pair) runs in bf16 split
    across DVE/GpSimd.  Host fixes constant signs and assembles.
  - Layout: freqs on psum partitions, frames on the moving dim (N=512).
    Frames advance by 64 in stream index, so even/odd frames use separate
    64-shifted block layouts.  Units split by (signal, parity, frame-tile,
    re/im) so psum per unit = 4 one-bank tiles -> 2-deep ping-pong.
  - k=128 and k=384 (2 of 513 freqs) via host BLAS; bf16 outputs upcast.
"""

import numpy as np
import ml_dtypes

N_FFT = 1024
HOP = 256
B = 16
T = 480000
F = N_FFT // 2 + 1          # 513
PAD = N_FFT // 2            # 512
XP_LEN = T + 2 * PAD        # 481024
NF = (XP_LEN - N_FFT) // HOP + 1   # 1876 frames
NCORES = 8
B_PER_CORE = B // NCORES    # 2
NFE = NF // 2               # 938 frames per parity
NBC = 941                   # stream-layout columns
XS_LEN = XP_LEN // 4        # 120256 samples per stream
FT_SIZES = [512, 426]       # frame tiles per parity (sum=938)
HOST_KS = [128, 384]        # freqs computed on host BLAS

_CACHE = {}


def _build_nc(repeat=1):
    import concourse.mybir as mybir
    import concourse.tile as tile
    from concourse import bacc

    idt = mybir.dt.bfloat16
    f32 = mybir.dt.float32
    add = mybir.AluOpType.add
    sub = mybir.AluOpType.subtract
    mult = mybir.AluOpType.mult

    nc = bacc.Bacc("TRN2", target_bir_lowering=False, debug=False,
                   num_devices=NCORES)
    sig = nc.dram_tensor("sig", [128, B_PER_CORE, 2, 4, NBC], idt,
                         kind="ExternalInput")
    basis = nc.dram_tensor("basis", [128, 16, 128], idt,
                           kind="ExternalInput")
    out = nc.dram_tensor("out", [B_PER_CORE, 2, 8, 128, NFE], idt,
                         kind="ExternalOutput")

    with tile.TileContext(nc) as tc:
        with (
            tc.tile_pool(name="sigp", bufs=min(repeat, 2)) as sigp,
            tc.tile_pool(name="basp", bufs=min(repeat, 2)) as basp,
            tc.tile_pool(name="intp", bufs=2) as intp,
            tc.tile_pool(name="outp", bufs=3) as outp,
            tc.tile_pool(name="ps", bufs=2, space="PSUM") as psp,
        ):
            for _rep in range(repeat):
                sg = sigp.tile([128, B_PER_CORE, 2, 4, NBC], idt,
                               name="sg", tag="sg")
                bs = basp.tile([128, 16, 128], idt, name="bs", tag="bs")
                nc.sync.dma_start(bs[:], basis[:])
                # head of (b0, par0) first so unit 0 starts ASAP
                nc.sync.dma_start(sg[:, 0, 0, :, 0:516], sig[:, 0, 0, :, 0:516])
                nc.sync.dma_start(sg[:, 0, 0, :, 516:], sig[:, 0, 0, :, 516:])
                nc.sync.dma_start(sg[:, 0, 1], sig[:, 0, 1])
                for b in range(1, B_PER_CORE):
                    for par in range(2):
                        nc.sync.dma_start(sg[:, b, par], sig[:, b, par])

                gp, ve, sc = nc.gpsimd, nc.vector, nc.scalar
                for b in range(B_PER_CORE):
                    for par in range(2):
                        for ft, N in enumerate(FT_SIZES):
                            f0 = 512 * ft
                            PM = {}
                            for comp in range(2):
                                # 4 psum tiles, S2/S3 first (ACT copies them)
                                S = [psp.tile([128, 512], f32, tag=f"S{c}",
                                              name=f"S{c}_{comp}")[:, 0:N]
                                     for c in range(4)]
                                for c in (2, 3, 0, 1):
                                    for q in range(2):
                                        ch = (c * 2 + comp) * 2 + q
                                        nc.tensor.matmul(
                                            S[c], bs[:, ch, :],
                                            sg[:, b, par, c, f0 + q:f0 + q + N],
                                            start=(q == 0), stop=(q == 1),
                                        )
                                it = {
                                    nm: intp.tile([128, 512], idt,
                                                  tag=f"{nm}{comp}",
                                                  name=f"{nm}{comp}")[:, 0:N]
                                    for nm in ("C2", "C3", "P02", "M02",
                                               "P13", "M13")
                                }
                                sc.mul(it["C2"], S[2], 2.0)
                                sc.mul(it["C3"], S[3], 2.0)
                                ve.scalar_tensor_tensor(
                                    it["P02"], it["C2"], 0.5, S[0], mult, add)
                                ve.scalar_tensor_tensor(
                                    it["P13"], it["C3"], 0.5, S[1], mult, add)
                                gp.tensor_tensor(it["M02"], it["C2"],
                                                 it["P02"], sub)
                                gp.tensor_tensor(it["M13"], it["C3"],
                                                 it["P13"], sub)
                                PM[comp] = it
                            r, i = PM[0], PM[1]
                            ot = [outp.tile([128, 512], idt, tag=f"o{j}",
                                            name=f"o{j}")[:, 0:N]
                                  for j in range(8)]
                            # W_re, W_im, Z_re, Z_im, U_re, U_im, V_re, V_im
                            ve.tensor_tensor(ot[0], r["P02"], r["P13"], add)
                            gp.tensor_tensor(ot[1], i["P02"], i["P13"], add)
                            ve.tensor_tensor(ot[2], r["P02"], r["P13"], sub)
                            gp.tensor_tensor(ot[3], i["P02"], i["P13"], sub)
                            ve.tensor_tensor(ot[4], r["M02"], i["M13"], add)
                            gp.tensor_tensor(ot[5], i["M02"], r["M13"], add)
                            ve.tensor_tensor(ot[6], r["M02"], i["M13"], sub)
                            ve.tensor_tensor(ot[7], i["M02"], r["M13"], sub)
                            for j in range(8):
                                nc.sync.dma_start(
                                    out[b, par, j, :, f0:f0 + N], ot[j])

    nc.compile()
    return nc


def _host_prep(x, window):
    x = np.asarray(x, dtype=np.float32)
    window = np.asarray(window, dtype=np.float32)
    xp = np.pad(x, ((0, 0), (PAD, PAD)), mode="reflect")

    # stream layouts: [B, 2par, 4c, 128, NBC]
    lay = np.zeros((B, 2, 4, 128, NBC), np.float32)
    for c in range(4):
        xs = xp[:, c::4]                              # [B, 120256]
        xs_pad = np.zeros((B, 64 + NBC * 128), np.float32)
        xs_pad[:, :XS_LEN] = xs
        lay[:, 0, c] = xs_pad[:, :NBC * 128].reshape(B, NBC, 128).transpose(0, 2, 1)
        lay[:, 1, c] = xs_pad[:, 64:64 + NBC * 128].reshape(B, NBC, 128).transpose(0, 2, 1)
    lay_bf = lay.astype(ml_dtypes.bfloat16)

    # Windowed DFT basis, fp32 angles to match the reference math.
    k = np.arange(F, dtype=np.float32)[:, None]
    n = np.arange(N_FFT, dtype=np.float32)[None, :]
    ang = np.float32(2.0 * np.pi / N_FFT) * k * n
    cosk = np.cos(ang) * window[None, :]              # [F, 1024]
    sink = -np.sin(ang) * window[None, :]
    Bre = cosk.T.astype(np.float32)                   # [1024, F]
    Bim = sink.T.astype(np.float32)

    bas_host = np.empty((128, 16, 128), np.float32)
    p = np.arange(128)
    for c in range(4):
        for comp in range(2):
            src = Bre if comp == 0 else Bim
            for q in range(2):
                ch = (c * 2 + comp) * 2 + q
                bas_host[:, ch, :] = src[4 * (128 * q + p) + c, :128]
    bas_bf = bas_host.astype(ml_dtypes.bfloat16)

    # host freqs k=128, 384 (fp32 BLAS)
    hb = np.stack([Bre[:, HOST_KS[0]], Bim[:, HOST_KS[0]],
                   Bre[:, HOST_KS[1]], Bim[:, HOST_KS[1]]], axis=1)  # [1024,4]
    hout = np.empty((B, NF, 4), np.float32)
    for b in range(B):
        frames = np.lib.stride_tricks.as_strided(
            xp[b], (NF, N_FFT), (HOP * 4, 4))
        hout[b] = frames @ hb

    in_maps = []
    for core in range(NCORES):
        bs = slice(B_PER_CORE * core, B_PER_CORE * (core + 1))
        sc = np.ascontiguousarray(lay_bf[bs].transpose(3, 0, 1, 2, 4))
        in_maps.append({"sig": sc, "basis": bas_bf})
    return in_maps, hout


def _assemble(results, prep):
    _, hout = prep
    dev = np.concatenate(
        [np.asarray(results.results[c]["out"]) for c in range(NCORES)],
        axis=0).astype(np.float32)                    # [B, 2, 8, 128, NFE]
    out = np.empty((B, NF, F, 2), np.float32)
    idx1 = 256 - np.arange(128)
    idx3 = 512 - np.arange(128)
    for par in range(2):
        A = dev[:, par].transpose(0, 3, 1, 2)         # [B, NFE, 8, 128]
        ov = out[:, par::2]                           # view [B, NFE, F, 2]
        # comps: 0 W_re 1 W_im 2 Z_re 3 Z_im 4 U_re 5 U_im 6 V_re 7 V_im
        ov[:, :, 0:128, 0] = A[:, :, 0]               # a0r
        ov[:, :, 0:128, 1] = A[:, :, 1]               # a0i
        ov[:, :, idx1, 0] = -A[:, :, 6]               # a1r = -V_re
        ov[:, :, idx1, 1] = A[:, :, 5]                # a1i = U_im
        ov[:, :, 256:384, 0] = -A[:, :, 4]            # a2r = -U_re
        ov[:, :, 256:384, 1] = -A[:, :, 7]            # a2i = -V_im
        ov[:, :, idx3, 0] = A[:, :, 2]                # a3r = Z_re
        ov[:, :, idx3, 1] = -A[:, :, 3]               # a3i = -Z_im
    out[:, :, HOST_KS[0], 0] = hout[:, :, 0]
    out[:, :, HOST_KS[0], 1] = hout[:, :, 1]
    out[:, :, HOST_KS[1], 0] = hout[:, :, 2]
    out[:, :, HOST_KS[1], 1] = hout[:, :, 3]
    return out


def kernel(x, window):
    from concourse.bass_utils import run_bass_kernel_spmd

    if "nc" not in _CACHE:
        _CACHE["nc"] = _build_nc()
    nc = _CACHE["nc"]

    prep = _host_prep(np.asarray(x), np.asarray(window))
    res = run_bass_kernel_spmd(nc, prep[0], core_ids=list(range(NCORES)),
                               trace=False)
    return _assemble(res, prep)
